# revision 26
# baseline (speedup 1.0000x reference)
"""MoC-SwiGLU (top-k channel masking) Trainium2 Bass kernel.

out = (topk_mask(silu(x@Wg.T) * (x@Wu.T), k=1024 by |z|)) @ Wd.T

Strategy: data-parallel over tokens across 8 NeuronCores. Host pre-transposes
and casts operands to fp16 (full PE speed, ~2.3x less quantization noise than
bf16 - selection flips near the top-k threshold dominate the error budget).
Per 128-token tile the top-k threshold comes from a 3-step Newton iteration
on the count function, t <- t*(1 + (count(|z|>=t)-K)/G), seeded at
1.0559*mean|z|; counting uses fused compare+reduce ops with tokens on
partitions, alternating tiles between DVE and ACT (Sign-with-bias trick) with
their per-step scalar updates interleaved so the two chains stay parallel.
The masked z is transposed by the DMA xbar (one dma_start_transpose per
tile; no PE identity matmuls, no PSUM->SBUF copies) in a 3-stage pipeline:
search, transpose-kick one superblock later (so the kickoff never blocks the
ACT queue), down-projection one superblock after that. DMA kickoffs are
spread across queues (weights on sync, x/Wd on gpsimd SWDGE, out on scalar)
because each engine queue is strictly in-order and a single late wait
head-of-line-blocks everything behind it.
"""

import numpy as np
import ml_dtypes

import concourse.bass as bass
import concourse.bacc as bacc
import concourse.mybir as mybir
import concourse.tile as tile
from concourse import masks
from concourse.bass_utils import run_bass_kernel_spmd

FP32 = mybir.dt.float32
FP16 = mybir.dt.float16
BF16 = mybir.dt.bfloat16
FP8 = mybir.dt.float8e4

# Problem geometry (full problem, hardcoded per the harness contract)
B, S, D = 4, 4096, 1024
F = 4096
K_ACTIVE = 1024
N_CORES = 8
TOKENS = B * S                    # 16384
TOK_CORE = TOKENS // N_CORES      # 2048


def _build_nc(tok_core=TOK_CORE, d=D, f=F, k_active=K_ACTIVE, sb=256, fb=512,
              niter=3, g_slope=1200.0, zmask2=None, debug=False,
              act_mod=2, act_rem=(1,),
              z_bufs=4, absz_bufs=2, zm_bufs=2, zt_bufs=2, w_bufs=4, x_bufs=2,
              out_bufs=1, s_bufs=3, gu_bufs=6, dn_bufs=2,
              init_lo=0.82 * 1.0559, init_hi=1.18 * 1.0559,
              delay_tiles=2, ind_bufs=1, wd_chunks=4,
              repeat=1):
    n_dc = d // 128
    n_fc = f // 128
    n_fb = f // fb
    n_sb = tok_core // sb
    tps = sb // 128

    nc = bacc.Bacc("TRN2", target_bir_lowering=False, debug=False)
    xT = nc.declare_dram_parameter("xT", [d, tok_core], FP16, isOutput=False)
    WgT = nc.declare_dram_parameter("WgT", [d, f], FP16, isOutput=False)
    WuT = nc.declare_dram_parameter("WuT", [d, f], FP16, isOutput=False)
    WdT = nc.declare_dram_parameter("WdT", [f, d], FP16, isOutput=False)
    out = nc.declare_dram_parameter("out", [tok_core, d], FP32, isOutput=True)
    if debug:
        z_dbg = nc.declare_dram_parameter("z_dbg", [tok_core, f], FP32, isOutput=True)
        lo_dbg = nc.declare_dram_parameter("lo_dbg", [tok_core, 1], FP32, isOutput=True)
        zm_dbg = nc.declare_dram_parameter("zm_dbg", [tok_core, f], FP32, isOutput=True)

    xT_r = xT.rearrange("(c p) t -> p c t", p=128)     # [128, n_dc, tok_core]
    WgT_r = WgT.rearrange("(c p) f -> p c f", p=128)   # [128, n_dc, f]
    WuT_r = WuT.rearrange("(c p) f -> p c f", p=128)
    WdT_r = WdT.rearrange("(c p) d -> p c d", p=128)   # [128, n_fc, d]

    with tile.TileContext(nc) as tc:
        with (
            tc.tile_pool(name="const", bufs=1) as const_pool,
            tc.tile_pool(name="wd", bufs=1) as wd_pool,
            tc.tile_pool(name="xs", bufs=x_bufs) as x_pool,
            tc.tile_pool(name="wgu", bufs=w_bufs) as w_pool,
            tc.tile_pool(name="zb", bufs=z_bufs) as z_pool,
            tc.tile_pool(name="absz", bufs=absz_bufs) as absz_pool,
            tc.tile_pool(name="zm", bufs=zm_bufs) as zm_pool,
            tc.tile_pool(name="indp", bufs=ind_bufs) as ind_pool,
            tc.tile_pool(name="ztr", bufs=zt_bufs) as zt_pool,
            tc.tile_pool(name="silu", bufs=s_bufs) as s_pool,
            tc.tile_pool(name="outp", bufs=out_bufs) as out_pool,
            tc.tile_pool(name="small", bufs=4) as sm_pool,
            tc.tile_pool(name="gu_ps", bufs=gu_bufs, space="PSUM") as gu_psum,
            tc.tile_pool(name="dn_ps", bufs=dn_bufs, space="PSUM") as dn_psum,
        ):
            wd_sb = wd_pool.tile([128, n_fc, d], FP16, tag="wd")
            wd_issued = 0
            fc_per_chunk = n_fc // wd_chunks
            if repeat > 1:
                nc.sync.dma_start(wd_sb[:], WdT_r[:])
                wd_issued = wd_chunks
                rep_cm = tc.For_i(0, repeat, 1)
                rep_cm.__enter__()

            tile_idx = 0
            searched = []
            transposed = []
            x_next = None
            for isb in range(n_sb):
                if x_next is not None:
                    x_sb = x_next
                else:
                    x_sb = x_pool.tile([128, n_dc, sb], FP16, tag="x")
                    # first block: sync queue ahead of all weight kickoffs,
                    # split so the first LDWEIGHTS waits on half the data
                    h = n_dc // 2
                    nc.sync.dma_start(x_sb[:, :h, :],
                                      xT_r[:, :h, isb * sb:(isb + 1) * sb])
                    nc.sync.dma_start(x_sb[:, h:, :],
                                      xT_r[:, h:, isb * sb:(isb + 1) * sb])

                z_tiles = [z_pool.tile([128, f], FP16, tag="z", name=f"z_{isb}_{i}")
                           for i in range(tps)]

                # kick last superblock's xbar transposes FIRST: their zmasks
                # are long complete, and ahead of this block's silus in the
                # ACT queue they run immediately -- so zt is ready well
                # before the down-projections that follow this block's MMs
                for (zmask_, lo_, zt_ref_, tok0_) in searched:
                    ztt = zt_pool.tile([128, n_fc, 128], FP16, tag="zt")
                    nc.scalar.dma_start_transpose(ztt[:], zmask_[:])
                    transposed.append((ztt, lo_, zt_ref_, tok0_))
                searched = []

                for ifb in range(n_fb):
                    wg_t = w_pool.tile([128, n_dc, fb], FP16, tag="w")
                    nc.sync.dma_start(wg_t[:], WgT_r[:, :, ifb * fb:(ifb + 1) * fb])
                    wu_t = w_pool.tile([128, n_dc, fb], FP16, tag="w")
                    nc.sync.dma_start(wu_t[:], WuT_r[:, :, ifb * fb:(ifb + 1) * fb])
                    gfb = isb * n_fb + ifb
                    if wd_issued < wd_chunks and gfb >= 8 and gfb % 2 == 0:
                        # Wd in chunks, spread past the startup-critical phase
                        # (HBM is weight-stream-bound for the first ~2
                        # superblocks); first use is ~2 superblocks in
                        ck = wd_issued
                        nc.gpsimd.dma_start(
                            wd_sb[:, ck * fc_per_chunk:(ck + 1) * fc_per_chunk, :],
                            WdT_r[:, ck * fc_per_chunk:(ck + 1) * fc_per_chunk, :])
                        wd_issued += 1
                    if ifb == 4 and isb + 1 < n_sb:
                        # prefetch next superblock's x mid-block so its first
                        # LDWEIGHTS never waits
                        x_next = x_pool.tile([128, n_dc, sb], FP16,
                                             tag="x")
                        nc.gpsimd.dma_start(
                            x_next[:], xT_r[:, :, (isb + 1) * sb:(isb + 2) * sb])

                    for tt in range(tps):
                        xw = x_sb[:, :, tt * 128:(tt + 1) * 128]
                        g_ps = gu_psum.tile([128, fb], FP32, tag="gu")
                        u_ps = gu_psum.tile([128, fb], FP32, tag="gu")
                        for dc in range(n_dc):
                            nc.tensor.matmul(g_ps[:], xw[:, dc, :], wg_t[:, dc, :],
                                             start=(dc == 0), stop=(dc == n_dc - 1))
                        for dc in range(n_dc):
                            nc.tensor.matmul(u_ps[:], xw[:, dc, :], wu_t[:, dc, :],
                                             start=(dc == 0), stop=(dc == n_dc - 1))
                        s_t = s_pool.tile([128, fb], FP16, tag="s")
                        nc.scalar.activation(s_t[:], g_ps[:],
                                             mybir.ActivationFunctionType.Silu)
                        nc.vector.tensor_tensor(
                            z_tiles[tt][:, ifb * fb:(ifb + 1) * fb],
                            s_t[:], u_ps[:], mybir.AluOpType.mult)

                def emit_search_group(z_list, tile_idx0):
                    """Search all tiles of this superblock with their Newton
                    steps interleaved across engines: ACT tiles run Sign on
                    the scalar engine, DVE tiles run the fused compare+reduce,
                    and the per-step smalls interleave on DVE so neither
                    tile's chain serializes behind the other's big ops."""
                    n = len(z_list)
                    on_act = [((tile_idx0 + j) % act_mod) in act_rem
                              for j in range(n)]
                    # ACT tiles' abs first: their Sign chains start earliest
                    order = sorted(range(n), key=lambda j: 0 if on_act[j] else 1)
                    st = [None] * n
                    for j in order:
                        absz = absz_pool.tile([128, f], FP16, tag="absz")
                        s1 = sm_pool.tile([128, 1], FP32, tag="s1")
                        nc.scalar.activation(absz[:], z_list[j][:],
                                             mybir.ActivationFunctionType.Abs,
                                             accum_out=s1[:, 0:1])
                        st[j] = {"absz": absz, "s1": s1}
                    for j in order:
                        lo = sm_pool.tile([128, 1], FP32, tag="lo")
                        dd = sm_pool.tile([128, 1], FP32, tag="dd")
                        cnt = sm_pool.tile([128, 1], FP32, tag="cnt")
                        sgn = -1.0 if on_act[j] else 1.0
                        nc.vector.tensor_scalar_mul(
                            lo[:], st[j]["s1"][:],
                            sgn * (init_lo + init_hi) / 2 / f)
                        ind = ind_pool.tile([128, f],
                                            FP8 if on_act[j] else FP16,
                                            tag="ind_a" if on_act[j] else "ind_v")
                        st[j].update(lo=lo, dd=dd, cnt=cnt, ind=ind)
                    # Newton: t <- t * (1 + (count(|z|>=t) - K)/G).
                    # ACT path tracks -t (Sign bias) and counts 2c - F.
                    for it in range(niter):
                        for j in order:
                            s = st[j]
                            if on_act[j]:
                                nc.scalar.activation(
                                    s["ind"][:], s["absz"][:],
                                    mybir.ActivationFunctionType.Sign,
                                    bias=s["lo"][:, 0:1],
                                    accum_out=s["cnt"][:, 0:1])
                            else:
                                nc.vector.tensor_scalar(
                                    s["ind"][:], s["absz"][:], s["lo"][:, 0:1],
                                    None, mybir.AluOpType.is_ge,
                                    mybir.AluOpType.add,
                                    accum_out=s["cnt"][:, 0:1])
                        for j in order:
                            s = st[j]
                            if on_act[j]:
                                nc.vector.tensor_scalar(
                                    s["dd"][:], s["cnt"][:],
                                    float(f - 2 * k_active),
                                    1.0 / (2 * g_slope),
                                    mybir.AluOpType.add, mybir.AluOpType.mult)
                            else:
                                nc.vector.tensor_scalar(
                                    s["dd"][:], s["cnt"][:], float(-k_active),
                                    1.0 / g_slope,
                                    mybir.AluOpType.add, mybir.AluOpType.mult)
                            nc.vector.tensor_single_scalar(
                                s["dd"][:], s["dd"][:], 1.0,
                                mybir.AluOpType.add)
                            nc.vector.tensor_tensor(
                                s["lo"][:], s["lo"][:], s["dd"][:],
                                mybir.AluOpType.mult)
                    res = [None] * n
                    for j in order:
                        s = st[j]
                        if on_act[j]:
                            nc.vector.tensor_scalar_mul(s["lo"][:], s["lo"][:],
                                                        -1.0)
                        zmask = zm_pool.tile([128, f], FP16, tag="zm")
                        if on_act[j]:
                            nc.vector.scalar_tensor_tensor(
                                zmask[:], s["absz"][:], s["lo"][:, 0:1],
                                z_list[j][:], mybir.AluOpType.is_ge,
                                mybir.AluOpType.mult)
                        else:
                            # 2-op fast path: 4x packed compare + 2x packed mult
                            nc.vector.tensor_scalar(
                                s["ind"][:], s["absz"][:], s["lo"][:, 0:1],
                                None, mybir.AluOpType.is_ge)
                            nc.vector.tensor_tensor(
                                zmask[:], s["ind"][:], z_list[j][:],
                                mybir.AluOpType.mult)
                        res[j] = (zmask, s["lo"], z_list[j])
                    return res

                def emit_td(zt_t, lo, z_t, tok0):
                    # down-projection: out[t, :] = sum_f zmask[t, f] * WdT[f, :]
                    out_t = out_pool.tile([128, d], FP32, tag="out")
                    dbw = min(512, d)
                    for db in range(d // dbw):
                        dn_ps = dn_psum.tile([128, dbw], FP32, tag="dn")
                        for c in range(n_fc):
                            nc.tensor.matmul(dn_ps[:], zt_t[:, c, :],
                                             wd_sb[:, c, db * dbw:(db + 1) * dbw],
                                             start=(c == 0), stop=(c == n_fc - 1))
                        nc.scalar.activation(out_t[:, db * dbw:(db + 1) * dbw],
                                             dn_ps[:],
                                             mybir.ActivationFunctionType.Copy)

                    # scalar-queue kickoff lands right after the producing
                    # copy with no cross-engine wait, and stays off the
                    # weight-stream queue
                    nc.scalar.dma_start(out[tok0:tok0 + 128, :], out_t[:])
                    if debug:
                        nc.sync.dma_start(lo_dbg[tok0:tok0 + 128, :], lo[:])
                        nc.gpsimd.dma_start(z_dbg[tok0:tok0 + 128, :], z_t[:])

                # down-project the tiles transposed one superblock ago,
                # then search this superblock
                while transposed and transposed[0][3] < (isb - 1) * sb:
                    (ztt, lo_, zt_ref_, tok0_) = transposed.pop(0)
                    emit_td(ztt, lo_, zt_ref_, tok0_)
                for tt, ctx_ in enumerate(emit_search_group(z_tiles, tile_idx)):
                    searched.append((*ctx_, isb * sb + tt * 128))
                tile_idx += tps
            while transposed:
                (ztt, lo_, zt_ref_, tok0_) = transposed.pop(0)
                emit_td(ztt, lo_, zt_ref_, tok0_)
            for (zmask_, lo_, zt_ref_, tok0_) in searched:
                ztt = zt_pool.tile([128, n_fc, 128], FP16, tag="zt")
                nc.scalar.dma_start_transpose(ztt[:], zmask_[:])
                emit_td(ztt, lo_, zt_ref_, tok0_)
            if repeat > 1:
                rep_cm.__exit__(None, None, None)
    nc.compile()
    return nc


_NC_CACHE = {}

# test-harness hooks (not used by the grading path)
TRACE = False
TRACE_KWARGS = {}
LAST_RESULT = None
BUILD_KWARGS = {}


def _get_nc(**kw):
    key = tuple(sorted(kw.items()))
    if key not in _NC_CACHE:
        _NC_CACHE[key] = _build_nc(**kw)
    return _NC_CACHE[key]


def kernel(x, Wg, Wu, Wd):
    xf = np.ascontiguousarray(x, dtype=np.float32).reshape(TOKENS, D)
    bf = np.float16
    WgT = np.ascontiguousarray(Wg.T).astype(bf)
    WuT = np.ascontiguousarray(Wu.T).astype(bf)
    WdT = np.ascontiguousarray(Wd.T).astype(bf)

    in_maps = []
    for c in range(N_CORES):
        xs = xf[c * TOK_CORE:(c + 1) * TOK_CORE]
        in_maps.append({
            "xT": np.ascontiguousarray(xs.T).astype(bf),
            "WgT": WgT, "WuT": WuT, "WdT": WdT,
        })

    nc = _get_nc(**BUILD_KWARGS)
    res = run_bass_kernel_spmd(nc, in_maps, core_ids=list(range(N_CORES)),
                               trace=TRACE, **TRACE_KWARGS)
    global LAST_RESULT
    LAST_RESULT = res
    out = np.concatenate([res.results[c]["out"] for c in range(N_CORES)], axis=0)
    return out.reshape(B, S, D)


# revision 27
# speedup vs baseline: 1.0208x; 1.0208x over previous
"""MoC-SwiGLU (top-k channel masking) Trainium2 Bass kernel.

out = (topk_mask(silu(x@Wg.T) * (x@Wu.T), k=1024 by |z|)) @ Wd.T

Strategy: data-parallel over tokens across 8 NeuronCores. Host pre-transposes
and casts operands to fp16 (full PE speed, ~2.3x less quantization noise than
bf16 - selection flips near the top-k threshold dominate the error budget).
Per 128-token tile the top-k threshold comes from a 3-step Newton iteration
on the count function, t <- t*(1 + (count(|z|>=t)-K)/G), seeded at
1.0559*mean|z|; counting uses fused compare+reduce ops with tokens on
partitions, alternating tiles between DVE and ACT (Sign-with-bias trick) with
their per-step scalar updates interleaved so the two chains stay parallel.
The masked z is transposed by the DMA xbar (one dma_start_transpose per
tile; no PE identity matmuls, no PSUM->SBUF copies) in a 3-stage pipeline:
search, transpose-kick one superblock later (so the kickoff never blocks the
ACT queue), down-projection one superblock after that. DMA kickoffs are
spread across queues (weights on sync, x/Wd on gpsimd SWDGE, out on scalar)
because each engine queue is strictly in-order and a single late wait
head-of-line-blocks everything behind it.
"""

import numpy as np
import ml_dtypes

import concourse.bass as bass
import concourse.bacc as bacc
import concourse.mybir as mybir
import concourse.tile as tile
from concourse import masks
from concourse.bass_utils import run_bass_kernel_spmd

FP32 = mybir.dt.float32
FP16 = mybir.dt.float16
BF16 = mybir.dt.bfloat16
FP8 = mybir.dt.float8e4

# Problem geometry (full problem, hardcoded per the harness contract)
B, S, D = 4, 4096, 1024
F = 4096
K_ACTIVE = 1024
N_CORES = 8
TOKENS = B * S                    # 16384
TOK_CORE = TOKENS // N_CORES      # 2048


def _build_nc(tok_core=TOK_CORE, d=D, f=F, k_active=K_ACTIVE, sb=256, fb=512,
              niter=3, g_slope=1200.0, zmask2=None, debug=False,
              act_mod=2, act_rem=(1,),
              z_bufs=4, absz_bufs=2, zm_bufs=2, zt_bufs=2, w_bufs=4, x_bufs=2,
              out_bufs=1, s_bufs=3, gu_bufs=6, dn_bufs=2,
              init_lo=0.82 * 1.0559, init_hi=1.18 * 1.0559,
              delay_tiles=2, ind_bufs=1, wd_chunks=4,
              repeat=1):
    n_dc = d // 128
    n_fc = f // 128
    n_fb = f // fb
    n_sb = tok_core // sb
    tps = sb // 128

    nc = bacc.Bacc("TRN2", target_bir_lowering=False, debug=False)
    xT = nc.declare_dram_parameter("xT", [d, tok_core], FP16, isOutput=False)
    WgT = nc.declare_dram_parameter("WgT", [d, f], FP16, isOutput=False)
    WuT = nc.declare_dram_parameter("WuT", [d, f], FP16, isOutput=False)
    WdT = nc.declare_dram_parameter("WdT", [f, d], FP16, isOutput=False)
    out = nc.declare_dram_parameter("out", [tok_core, d], FP32, isOutput=True)
    if debug:
        z_dbg = nc.declare_dram_parameter("z_dbg", [tok_core, f], FP32, isOutput=True)
        lo_dbg = nc.declare_dram_parameter("lo_dbg", [tok_core, 1], FP32, isOutput=True)
        zm_dbg = nc.declare_dram_parameter("zm_dbg", [tok_core, f], FP32, isOutput=True)

    xT_r = xT.rearrange("(c p) t -> p c t", p=128)     # [128, n_dc, tok_core]
    WgT_r = WgT.rearrange("(c p) f -> p c f", p=128)   # [128, n_dc, f]
    WuT_r = WuT.rearrange("(c p) f -> p c f", p=128)
    WdT_r = WdT.rearrange("(c p) d -> p c d", p=128)   # [128, n_fc, d]

    with tile.TileContext(nc) as tc:
        with (
            tc.tile_pool(name="const", bufs=1) as const_pool,
            tc.tile_pool(name="wd", bufs=1) as wd_pool,
            tc.tile_pool(name="xs", bufs=x_bufs) as x_pool,
            tc.tile_pool(name="wgu", bufs=w_bufs) as w_pool,
            tc.tile_pool(name="zb", bufs=z_bufs) as z_pool,
            tc.tile_pool(name="absz", bufs=absz_bufs) as absz_pool,
            tc.tile_pool(name="zm", bufs=zm_bufs) as zm_pool,
            tc.tile_pool(name="indp", bufs=ind_bufs) as ind_pool,
            tc.tile_pool(name="ztr", bufs=zt_bufs) as zt_pool,
            tc.tile_pool(name="silu", bufs=s_bufs) as s_pool,
            tc.tile_pool(name="outp", bufs=out_bufs) as out_pool,
            tc.tile_pool(name="small", bufs=4) as sm_pool,
            tc.tile_pool(name="gu_ps", bufs=gu_bufs, space="PSUM") as gu_psum,
            tc.tile_pool(name="dn_ps", bufs=dn_bufs, space="PSUM") as dn_psum,
        ):
            wd_sb = wd_pool.tile([128, n_fc, d], FP16, tag="wd")
            wd_issued = 0
            fc_per_chunk = n_fc // wd_chunks
            if repeat > 1:
                nc.sync.dma_start(wd_sb[:], WdT_r[:])
                wd_issued = wd_chunks
                rep_cm = tc.For_i(0, repeat, 1)
                rep_cm.__enter__()

            tile_idx = 0
            searched = []
            transposed = []
            x_next = None
            for isb in range(n_sb):
                if x_next is not None:
                    x_sb = x_next
                else:
                    x_sb = x_pool.tile([128, n_dc, sb], FP16, tag="x")
                    # first block: sync queue ahead of all weight kickoffs,
                    # split so the first LDWEIGHTS waits on half the data
                    h = n_dc // 2
                    nc.sync.dma_start(x_sb[:, :h, :],
                                      xT_r[:, :h, isb * sb:(isb + 1) * sb])
                    nc.sync.dma_start(x_sb[:, h:, :],
                                      xT_r[:, h:, isb * sb:(isb + 1) * sb])

                z_tiles = [z_pool.tile([128, f], FP16, tag="z", name=f"z_{isb}_{i}")
                           for i in range(tps)]

                for ifb in range(n_fb):
                    wg_t = w_pool.tile([128, n_dc, fb], FP16, tag="w")
                    nc.sync.dma_start(wg_t[:], WgT_r[:, :, ifb * fb:(ifb + 1) * fb])
                    wu_t = w_pool.tile([128, n_dc, fb], FP16, tag="w")
                    nc.sync.dma_start(wu_t[:], WuT_r[:, :, ifb * fb:(ifb + 1) * fb])
                    gfb = isb * n_fb + ifb
                    if wd_issued < wd_chunks and gfb >= 8 and gfb % 2 == 0:
                        # Wd in chunks, spread past the startup-critical phase
                        # (HBM is weight-stream-bound for the first ~2
                        # superblocks); first use is ~2 superblocks in
                        ck = wd_issued
                        nc.gpsimd.dma_start(
                            wd_sb[:, ck * fc_per_chunk:(ck + 1) * fc_per_chunk, :],
                            WdT_r[:, ck * fc_per_chunk:(ck + 1) * fc_per_chunk, :])
                        wd_issued += 1
                    if ifb == 4 and isb + 1 < n_sb:
                        # prefetch next superblock's x mid-block so its first
                        # LDWEIGHTS never waits
                        x_next = x_pool.tile([128, n_dc, sb], FP16,
                                             tag="x")
                        nc.gpsimd.dma_start(
                            x_next[:], xT_r[:, :, (isb + 1) * sb:(isb + 2) * sb])

                    for tt in range(tps):
                        xw = x_sb[:, :, tt * 128:(tt + 1) * 128]
                        g_ps = gu_psum.tile([128, fb], FP32, tag="gu")
                        u_ps = gu_psum.tile([128, fb], FP32, tag="gu")
                        for dc in range(n_dc):
                            nc.tensor.matmul(g_ps[:], xw[:, dc, :], wg_t[:, dc, :],
                                             start=(dc == 0), stop=(dc == n_dc - 1))
                        for dc in range(n_dc):
                            nc.tensor.matmul(u_ps[:], xw[:, dc, :], wu_t[:, dc, :],
                                             start=(dc == 0), stop=(dc == n_dc - 1))
                        s_t = s_pool.tile([128, fb], FP16, tag="s")
                        nc.scalar.activation(s_t[:], g_ps[:],
                                             mybir.ActivationFunctionType.Silu)
                        nc.vector.tensor_tensor(
                            z_tiles[tt][:, ifb * fb:(ifb + 1) * fb],
                            s_t[:], u_ps[:], mybir.AluOpType.mult)

                def emit_search_group(z_list, tile_idx0):
                    """Search all tiles of this superblock with their Newton
                    steps interleaved across engines: ACT tiles run Sign on
                    the scalar engine, DVE tiles run the fused compare+reduce,
                    and the per-step smalls interleave on DVE so neither
                    tile's chain serializes behind the other's big ops."""
                    n = len(z_list)
                    on_act = [((tile_idx0 + j) % act_mod) in act_rem
                              for j in range(n)]
                    # ACT tiles' abs first: their Sign chains start earliest
                    order = sorted(range(n), key=lambda j: 0 if on_act[j] else 1)
                    st = [None] * n
                    for j in order:
                        absz = absz_pool.tile([128, f], FP16, tag="absz")
                        s1 = sm_pool.tile([128, 1], FP32, tag="s1")
                        nc.scalar.activation(absz[:], z_list[j][:],
                                             mybir.ActivationFunctionType.Abs,
                                             accum_out=s1[:, 0:1])
                        st[j] = {"absz": absz, "s1": s1}
                    for j in order:
                        lo = sm_pool.tile([128, 1], FP32, tag="lo")
                        dd = sm_pool.tile([128, 1], FP32, tag="dd")
                        cnt = sm_pool.tile([128, 1], FP32, tag="cnt")
                        sgn = -1.0 if on_act[j] else 1.0
                        nc.vector.tensor_scalar_mul(
                            lo[:], st[j]["s1"][:],
                            sgn * (init_lo + init_hi) / 2 / f)
                        ind = ind_pool.tile([128, f],
                                            FP8 if on_act[j] else FP16,
                                            tag="ind_a" if on_act[j] else "ind_v")
                        st[j].update(lo=lo, dd=dd, cnt=cnt, ind=ind)
                    # Newton: t <- t * (1 + (count(|z|>=t) - K)/G).
                    # ACT path tracks -t (Sign bias) and counts 2c - F.
                    for it in range(niter):
                        for j in order:
                            s = st[j]
                            if on_act[j]:
                                nc.scalar.activation(
                                    s["ind"][:], s["absz"][:],
                                    mybir.ActivationFunctionType.Sign,
                                    bias=s["lo"][:, 0:1],
                                    accum_out=s["cnt"][:, 0:1])
                            else:
                                nc.vector.tensor_scalar(
                                    s["ind"][:], s["absz"][:], s["lo"][:, 0:1],
                                    None, mybir.AluOpType.is_ge,
                                    mybir.AluOpType.add,
                                    accum_out=s["cnt"][:, 0:1])
                        for j in order:
                            s = st[j]
                            if on_act[j]:
                                nc.vector.tensor_scalar(
                                    s["dd"][:], s["cnt"][:],
                                    float(f - 2 * k_active),
                                    1.0 / (2 * g_slope),
                                    mybir.AluOpType.add, mybir.AluOpType.mult)
                            else:
                                nc.vector.tensor_scalar(
                                    s["dd"][:], s["cnt"][:], float(-k_active),
                                    1.0 / g_slope,
                                    mybir.AluOpType.add, mybir.AluOpType.mult)
                            nc.vector.tensor_single_scalar(
                                s["dd"][:], s["dd"][:], 1.0,
                                mybir.AluOpType.add)
                            nc.vector.tensor_tensor(
                                s["lo"][:], s["lo"][:], s["dd"][:],
                                mybir.AluOpType.mult)
                    res = [None] * n
                    for j in order:
                        s = st[j]
                        if on_act[j]:
                            nc.vector.tensor_scalar_mul(s["lo"][:], s["lo"][:],
                                                        -1.0)
                        zmask = zm_pool.tile([128, f], FP16, tag="zm")
                        if on_act[j]:
                            nc.vector.scalar_tensor_tensor(
                                zmask[:], s["absz"][:], s["lo"][:, 0:1],
                                z_list[j][:], mybir.AluOpType.is_ge,
                                mybir.AluOpType.mult)
                        else:
                            # 2-op fast path: 4x packed compare + 2x packed mult
                            nc.vector.tensor_scalar(
                                s["ind"][:], s["absz"][:], s["lo"][:, 0:1],
                                None, mybir.AluOpType.is_ge)
                            nc.vector.tensor_tensor(
                                zmask[:], s["ind"][:], z_list[j][:],
                                mybir.AluOpType.mult)
                        res[j] = (zmask, s["lo"], z_list[j])
                    return res

                def emit_td(zt_t, lo, z_t, tok0):
                    # down-projection: out[t, :] = sum_f zmask[t, f] * WdT[f, :]
                    out_t = out_pool.tile([128, d], FP32, tag="out")
                    dbw = min(512, d)
                    for db in range(d // dbw):
                        dn_ps = dn_psum.tile([128, dbw], FP32, tag="dn")
                        for c in range(n_fc):
                            nc.tensor.matmul(dn_ps[:], zt_t[:, c, :],
                                             wd_sb[:, c, db * dbw:(db + 1) * dbw],
                                             start=(c == 0), stop=(c == n_fc - 1))
                        nc.scalar.activation(out_t[:, db * dbw:(db + 1) * dbw],
                                             dn_ps[:],
                                             mybir.ActivationFunctionType.Copy)

                    # scalar-queue kickoff lands right after the producing
                    # copy with no cross-engine wait, and stays off the
                    # weight-stream queue
                    nc.scalar.dma_start(out[tok0:tok0 + 128, :], out_t[:])
                    if debug:
                        nc.sync.dma_start(lo_dbg[tok0:tok0 + 128, :], lo[:])
                        nc.gpsimd.dma_start(z_dbg[tok0:tok0 + 128, :], z_t[:])

                # three-stage pipeline, one superblock apart each: kick
                # the xbar transposes for last superblock's zmasks (complete
                # by now, so the kickoff never blocks the ACT queue; placed
                # here, after the silus, their xbar traffic also stays out of
                # the weight stream's bandwidth-critical phase), down-project
                # the tiles transposed one superblock before that, then
                # search this superblock.
                for (zmask_, lo_, zt_ref_, tok0_) in searched:
                    ztt = zt_pool.tile([128, n_fc, 128], FP16, tag="zt")
                    nc.scalar.dma_start_transpose(ztt[:], zmask_[:])
                    transposed.append((ztt, lo_, zt_ref_, tok0_))
                searched = []
                while transposed and transposed[0][3] < (isb - 1) * sb:
                    (ztt, lo_, zt_ref_, tok0_) = transposed.pop(0)
                    emit_td(ztt, lo_, zt_ref_, tok0_)
                for tt, ctx_ in enumerate(emit_search_group(z_tiles, tile_idx)):
                    searched.append((*ctx_, isb * sb + tt * 128))
                tile_idx += tps
            while transposed:
                (ztt, lo_, zt_ref_, tok0_) = transposed.pop(0)
                emit_td(ztt, lo_, zt_ref_, tok0_)
            for (zmask_, lo_, zt_ref_, tok0_) in searched:
                ztt = zt_pool.tile([128, n_fc, 128], FP16, tag="zt")
                nc.scalar.dma_start_transpose(ztt[:], zmask_[:])
                emit_td(ztt, lo_, zt_ref_, tok0_)
            if repeat > 1:
                rep_cm.__exit__(None, None, None)
    nc.compile()
    return nc


_NC_CACHE = {}

# test-harness hooks (not used by the grading path)
TRACE = False
TRACE_KWARGS = {}
LAST_RESULT = None
BUILD_KWARGS = {}


def _get_nc(**kw):
    key = tuple(sorted(kw.items()))
    if key not in _NC_CACHE:
        _NC_CACHE[key] = _build_nc(**kw)
    return _NC_CACHE[key]


def kernel(x, Wg, Wu, Wd):
    xf = np.ascontiguousarray(x, dtype=np.float32).reshape(TOKENS, D)
    bf = np.float16
    WgT = np.ascontiguousarray(Wg.T).astype(bf)
    WuT = np.ascontiguousarray(Wu.T).astype(bf)
    WdT = np.ascontiguousarray(Wd.T).astype(bf)

    in_maps = []
    for c in range(N_CORES):
        xs = xf[c * TOK_CORE:(c + 1) * TOK_CORE]
        in_maps.append({
            "xT": np.ascontiguousarray(xs.T).astype(bf),
            "WgT": WgT, "WuT": WuT, "WdT": WdT,
        })

    nc = _get_nc(**BUILD_KWARGS)
    res = run_bass_kernel_spmd(nc, in_maps, core_ids=list(range(N_CORES)),
                               trace=TRACE, **TRACE_KWARGS)
    global LAST_RESULT
    LAST_RESULT = res
    out = np.concatenate([res.results[c]["out"] for c in range(N_CORES)], axis=0)
    return out.reshape(B, S, D)


# revision 28
# speedup vs baseline: 1.0534x; 1.0319x over previous
"""MoC-SwiGLU (top-k channel masking) Trainium2 Bass kernel.

out = (topk_mask(silu(x@Wg.T) * (x@Wu.T), k=1024 by |z|)) @ Wd.T

Strategy: data-parallel over tokens across 8 NeuronCores. Host pre-transposes
and casts operands to fp16 (full PE speed, ~2.3x less quantization noise than
bf16 - selection flips near the top-k threshold dominate the error budget).
Per 128-token tile the top-k threshold comes from a 3-step Newton iteration
on the count function, t <- t*(1 + (count(|z|>=t)-K)/G), seeded at
1.0559*mean|z|; counting uses fused compare+reduce ops with tokens on
partitions, alternating tiles between DVE and ACT (Sign-with-bias trick) with
their per-step scalar updates interleaved so the two chains stay parallel.
The masked z is transposed by the DMA xbar (one dma_start_transpose per
tile; no PE identity matmuls, no PSUM->SBUF copies) in a 3-stage pipeline:
search, transpose-kick one superblock later (so the kickoff never blocks the
ACT queue), down-projection one superblock after that. DMA kickoffs are
spread across queues (weights on sync, x/Wd on gpsimd SWDGE, out on scalar)
because each engine queue is strictly in-order and a single late wait
head-of-line-blocks everything behind it.
"""

import numpy as np
import ml_dtypes

import concourse.bass as bass
import concourse.bacc as bacc
import concourse.mybir as mybir
import concourse.tile as tile
from concourse import masks
from concourse.bass_utils import run_bass_kernel_spmd

FP32 = mybir.dt.float32
FP16 = mybir.dt.float16
BF16 = mybir.dt.bfloat16
FP8 = mybir.dt.float8e4

# Problem geometry (full problem, hardcoded per the harness contract)
B, S, D = 4, 4096, 1024
F = 4096
K_ACTIVE = 1024
N_CORES = 8
TOKENS = B * S                    # 16384
TOK_CORE = TOKENS // N_CORES      # 2048


def _build_nc(tok_core=TOK_CORE, d=D, f=F, k_active=K_ACTIVE, sb=256, fb=512,
              niter=2, g_slope=1100.0, zmask2=None, debug=False,
              act_mod=2, act_rem=(1,),
              z_bufs=4, absz_bufs=2, zm_bufs=2, zt_bufs=2, w_bufs=4, x_bufs=2,
              out_bufs=1, s_bufs=3, gu_bufs=6, dn_bufs=2,
              init_lo=0.82 * 1.0559, init_hi=1.18 * 1.0559,
              delay_tiles=2, ind_bufs=1, wd_chunks=4,
              repeat=1):
    n_dc = d // 128
    n_fc = f // 128
    n_fb = f // fb
    n_sb = tok_core // sb
    tps = sb // 128

    nc = bacc.Bacc("TRN2", target_bir_lowering=False, debug=False)
    xT = nc.declare_dram_parameter("xT", [d, tok_core], FP16, isOutput=False)
    WgT = nc.declare_dram_parameter("WgT", [d, f], FP16, isOutput=False)
    WuT = nc.declare_dram_parameter("WuT", [d, f], FP16, isOutput=False)
    WdT = nc.declare_dram_parameter("WdT", [f, d], FP16, isOutput=False)
    out = nc.declare_dram_parameter("out", [tok_core, d], FP32, isOutput=True)
    if debug:
        z_dbg = nc.declare_dram_parameter("z_dbg", [tok_core, f], FP32, isOutput=True)
        lo_dbg = nc.declare_dram_parameter("lo_dbg", [tok_core, 1], FP32, isOutput=True)
        zm_dbg = nc.declare_dram_parameter("zm_dbg", [tok_core, f], FP32, isOutput=True)

    xT_r = xT.rearrange("(c p) t -> p c t", p=128)     # [128, n_dc, tok_core]
    WgT_r = WgT.rearrange("(c p) f -> p c f", p=128)   # [128, n_dc, f]
    WuT_r = WuT.rearrange("(c p) f -> p c f", p=128)
    WdT_r = WdT.rearrange("(c p) d -> p c d", p=128)   # [128, n_fc, d]

    with tile.TileContext(nc) as tc:
        with (
            tc.tile_pool(name="const", bufs=1) as const_pool,
            tc.tile_pool(name="wd", bufs=1) as wd_pool,
            tc.tile_pool(name="xs", bufs=x_bufs) as x_pool,
            tc.tile_pool(name="wgu", bufs=w_bufs) as w_pool,
            tc.tile_pool(name="zb", bufs=z_bufs) as z_pool,
            tc.tile_pool(name="absz", bufs=absz_bufs) as absz_pool,
            tc.tile_pool(name="zm", bufs=zm_bufs) as zm_pool,
            tc.tile_pool(name="indp", bufs=ind_bufs) as ind_pool,
            tc.tile_pool(name="ztr", bufs=zt_bufs) as zt_pool,
            tc.tile_pool(name="silu", bufs=s_bufs) as s_pool,
            tc.tile_pool(name="outp", bufs=out_bufs) as out_pool,
            tc.tile_pool(name="small", bufs=4) as sm_pool,
            tc.tile_pool(name="gu_ps", bufs=gu_bufs, space="PSUM") as gu_psum,
            tc.tile_pool(name="dn_ps", bufs=dn_bufs, space="PSUM") as dn_psum,
        ):
            wd_sb = wd_pool.tile([128, n_fc, d], FP16, tag="wd")
            wd_issued = 0
            fc_per_chunk = n_fc // wd_chunks
            if repeat > 1:
                nc.sync.dma_start(wd_sb[:], WdT_r[:])
                wd_issued = wd_chunks
                rep_cm = tc.For_i(0, repeat, 1)
                rep_cm.__enter__()

            tile_idx = 0
            searched = []
            transposed = []
            x_next = None
            for isb in range(n_sb):
                if x_next is not None:
                    x_sb = x_next
                else:
                    x_sb = x_pool.tile([128, n_dc, sb], FP16, tag="x")
                    # first block: sync queue ahead of all weight kickoffs,
                    # split so the first LDWEIGHTS waits on half the data
                    h = n_dc // 2
                    nc.sync.dma_start(x_sb[:, :h, :],
                                      xT_r[:, :h, isb * sb:(isb + 1) * sb])
                    nc.sync.dma_start(x_sb[:, h:, :],
                                      xT_r[:, h:, isb * sb:(isb + 1) * sb])

                z_tiles = [z_pool.tile([128, f], FP16, tag="z", name=f"z_{isb}_{i}")
                           for i in range(tps)]

                for ifb in range(n_fb):
                    wg_t = w_pool.tile([128, n_dc, fb], FP16, tag="w")
                    nc.sync.dma_start(wg_t[:], WgT_r[:, :, ifb * fb:(ifb + 1) * fb])
                    wu_t = w_pool.tile([128, n_dc, fb], FP16, tag="w")
                    nc.sync.dma_start(wu_t[:], WuT_r[:, :, ifb * fb:(ifb + 1) * fb])
                    gfb = isb * n_fb + ifb
                    if wd_issued < wd_chunks and gfb >= 8 and gfb % 2 == 0:
                        # Wd in chunks, spread past the startup-critical phase
                        # (HBM is weight-stream-bound for the first ~2
                        # superblocks); first use is ~2 superblocks in
                        ck = wd_issued
                        nc.gpsimd.dma_start(
                            wd_sb[:, ck * fc_per_chunk:(ck + 1) * fc_per_chunk, :],
                            WdT_r[:, ck * fc_per_chunk:(ck + 1) * fc_per_chunk, :])
                        wd_issued += 1
                    if ifb == 4 and isb + 1 < n_sb:
                        # prefetch next superblock's x mid-block so its first
                        # LDWEIGHTS never waits
                        x_next = x_pool.tile([128, n_dc, sb], FP16,
                                             tag="x")
                        nc.gpsimd.dma_start(
                            x_next[:], xT_r[:, :, (isb + 1) * sb:(isb + 2) * sb])

                    for tt in range(tps):
                        xw = x_sb[:, :, tt * 128:(tt + 1) * 128]
                        g_ps = gu_psum.tile([128, fb], FP32, tag="gu")
                        u_ps = gu_psum.tile([128, fb], FP32, tag="gu")
                        for dc in range(n_dc):
                            nc.tensor.matmul(g_ps[:], xw[:, dc, :], wg_t[:, dc, :],
                                             start=(dc == 0), stop=(dc == n_dc - 1))
                        for dc in range(n_dc):
                            nc.tensor.matmul(u_ps[:], xw[:, dc, :], wu_t[:, dc, :],
                                             start=(dc == 0), stop=(dc == n_dc - 1))
                        s_t = s_pool.tile([128, fb], FP16, tag="s")
                        nc.scalar.activation(s_t[:], g_ps[:],
                                             mybir.ActivationFunctionType.Silu)
                        nc.vector.tensor_tensor(
                            z_tiles[tt][:, ifb * fb:(ifb + 1) * fb],
                            s_t[:], u_ps[:], mybir.AluOpType.mult)

                def emit_search_group(z_list, tile_idx0):
                    """Search all tiles of this superblock with their Newton
                    steps interleaved across engines: ACT tiles run Sign on
                    the scalar engine, DVE tiles run the fused compare+reduce,
                    and the per-step smalls interleave on DVE so neither
                    tile's chain serializes behind the other's big ops."""
                    n = len(z_list)
                    on_act = [((tile_idx0 + j) % act_mod) in act_rem
                              for j in range(n)]
                    # ACT tiles' abs first: their Sign chains start earliest
                    order = sorted(range(n), key=lambda j: 0 if on_act[j] else 1)
                    st = [None] * n
                    for j in order:
                        absz = absz_pool.tile([128, f], FP16, tag="absz")
                        s1 = sm_pool.tile([128, 1], FP32, tag="s1")
                        nc.scalar.activation(absz[:], z_list[j][:],
                                             mybir.ActivationFunctionType.Abs,
                                             accum_out=s1[:, 0:1])
                        st[j] = {"absz": absz, "s1": s1}
                    for j in order:
                        lo = sm_pool.tile([128, 1], FP32, tag="lo")
                        dd = sm_pool.tile([128, 1], FP32, tag="dd")
                        cnt = sm_pool.tile([128, 1], FP32, tag="cnt")
                        sgn = -1.0 if on_act[j] else 1.0
                        nc.vector.tensor_scalar_mul(
                            lo[:], st[j]["s1"][:],
                            sgn * (init_lo + init_hi) / 2 / f)
                        ind = ind_pool.tile([128, f],
                                            FP8 if on_act[j] else FP16,
                                            tag="ind_a" if on_act[j] else "ind_v")
                        st[j].update(lo=lo, dd=dd, cnt=cnt, ind=ind)
                    # Newton: t <- t * (1 + (count(|z|>=t) - K)/G).
                    # ACT path tracks -t (Sign bias) and counts 2c - F.
                    for it in range(niter):
                        for j in order:
                            s = st[j]
                            if on_act[j]:
                                nc.scalar.activation(
                                    s["ind"][:], s["absz"][:],
                                    mybir.ActivationFunctionType.Sign,
                                    bias=s["lo"][:, 0:1],
                                    accum_out=s["cnt"][:, 0:1])
                            else:
                                nc.vector.tensor_scalar(
                                    s["ind"][:], s["absz"][:], s["lo"][:, 0:1],
                                    None, mybir.AluOpType.is_ge,
                                    mybir.AluOpType.add,
                                    accum_out=s["cnt"][:, 0:1])
                        for j in order:
                            s = st[j]
                            if on_act[j]:
                                nc.vector.tensor_scalar(
                                    s["dd"][:], s["cnt"][:],
                                    float(f - 2 * k_active),
                                    1.0 / (2 * g_slope),
                                    mybir.AluOpType.add, mybir.AluOpType.mult)
                            else:
                                nc.vector.tensor_scalar(
                                    s["dd"][:], s["cnt"][:], float(-k_active),
                                    1.0 / g_slope,
                                    mybir.AluOpType.add, mybir.AluOpType.mult)
                            nc.vector.tensor_single_scalar(
                                s["dd"][:], s["dd"][:], 1.0,
                                mybir.AluOpType.add)
                            nc.vector.tensor_tensor(
                                s["lo"][:], s["lo"][:], s["dd"][:],
                                mybir.AluOpType.mult)
                    res = [None] * n
                    for j in order:
                        s = st[j]
                        if on_act[j]:
                            nc.vector.tensor_scalar_mul(s["lo"][:], s["lo"][:],
                                                        -1.0)
                        zmask = zm_pool.tile([128, f], FP16, tag="zm")
                        if on_act[j]:
                            nc.vector.scalar_tensor_tensor(
                                zmask[:], s["absz"][:], s["lo"][:, 0:1],
                                z_list[j][:], mybir.AluOpType.is_ge,
                                mybir.AluOpType.mult)
                        else:
                            # 2-op fast path: 4x packed compare + 2x packed mult
                            nc.vector.tensor_scalar(
                                s["ind"][:], s["absz"][:], s["lo"][:, 0:1],
                                None, mybir.AluOpType.is_ge)
                            nc.vector.tensor_tensor(
                                zmask[:], s["ind"][:], z_list[j][:],
                                mybir.AluOpType.mult)
                        res[j] = (zmask, s["lo"], z_list[j])
                    return res

                def emit_td(zt_t, lo, z_t, tok0):
                    # down-projection: out[t, :] = sum_f zmask[t, f] * WdT[f, :]
                    out_t = out_pool.tile([128, d], FP32, tag="out")
                    dbw = min(512, d)
                    for db in range(d // dbw):
                        dn_ps = dn_psum.tile([128, dbw], FP32, tag="dn")
                        for c in range(n_fc):
                            nc.tensor.matmul(dn_ps[:], zt_t[:, c, :],
                                             wd_sb[:, c, db * dbw:(db + 1) * dbw],
                                             start=(c == 0), stop=(c == n_fc - 1))
                        nc.scalar.activation(out_t[:, db * dbw:(db + 1) * dbw],
                                             dn_ps[:],
                                             mybir.ActivationFunctionType.Copy)

                    # scalar-queue kickoff lands right after the producing
                    # copy with no cross-engine wait, and stays off the
                    # weight-stream queue
                    nc.scalar.dma_start(out[tok0:tok0 + 128, :], out_t[:])
                    if debug:
                        nc.sync.dma_start(lo_dbg[tok0:tok0 + 128, :], lo[:])
                        nc.gpsimd.dma_start(z_dbg[tok0:tok0 + 128, :], z_t[:])

                # three-stage pipeline, one superblock apart each: kick
                # the xbar transposes for last superblock's zmasks (complete
                # by now, so the kickoff never blocks the ACT queue; placed
                # here, after the silus, their xbar traffic also stays out of
                # the weight stream's bandwidth-critical phase), down-project
                # the tiles transposed one superblock before that, then
                # search this superblock.
                for (zmask_, lo_, zt_ref_, tok0_) in searched:
                    ztt = zt_pool.tile([128, n_fc, 128], FP16, tag="zt")
                    nc.scalar.dma_start_transpose(ztt[:], zmask_[:])
                    transposed.append((ztt, lo_, zt_ref_, tok0_))
                searched = []
                while transposed and transposed[0][3] < (isb - 1) * sb:
                    (ztt, lo_, zt_ref_, tok0_) = transposed.pop(0)
                    emit_td(ztt, lo_, zt_ref_, tok0_)
                for tt, ctx_ in enumerate(emit_search_group(z_tiles, tile_idx)):
                    searched.append((*ctx_, isb * sb + tt * 128))
                tile_idx += tps
            while transposed:
                (ztt, lo_, zt_ref_, tok0_) = transposed.pop(0)
                emit_td(ztt, lo_, zt_ref_, tok0_)
            for (zmask_, lo_, zt_ref_, tok0_) in searched:
                ztt = zt_pool.tile([128, n_fc, 128], FP16, tag="zt")
                nc.scalar.dma_start_transpose(ztt[:], zmask_[:])
                emit_td(ztt, lo_, zt_ref_, tok0_)
            if repeat > 1:
                rep_cm.__exit__(None, None, None)
    nc.compile()
    return nc


_NC_CACHE = {}

# test-harness hooks (not used by the grading path)
TRACE = False
TRACE_KWARGS = {}
LAST_RESULT = None
BUILD_KWARGS = {}


def _get_nc(**kw):
    key = tuple(sorted(kw.items()))
    if key not in _NC_CACHE:
        _NC_CACHE[key] = _build_nc(**kw)
    return _NC_CACHE[key]


def kernel(x, Wg, Wu, Wd):
    xf = np.ascontiguousarray(x, dtype=np.float32).reshape(TOKENS, D)
    bf = np.float16
    WgT = np.ascontiguousarray(Wg.T).astype(bf)
    WuT = np.ascontiguousarray(Wu.T).astype(bf)
    WdT = np.ascontiguousarray(Wd.T).astype(bf)

    in_maps = []
    for c in range(N_CORES):
        xs = xf[c * TOK_CORE:(c + 1) * TOK_CORE]
        in_maps.append({
            "xT": np.ascontiguousarray(xs.T).astype(bf),
            "WgT": WgT, "WuT": WuT, "WdT": WdT,
        })

    nc = _get_nc(**BUILD_KWARGS)
    res = run_bass_kernel_spmd(nc, in_maps, core_ids=list(range(N_CORES)),
                               trace=TRACE, **TRACE_KWARGS)
    global LAST_RESULT
    LAST_RESULT = res
    out = np.concatenate([res.results[c]["out"] for c in range(N_CORES)], axis=0)
    return out.reshape(B, S, D)


# revision 30
# speedup vs baseline: 1.0565x; 1.0029x over previous
"""MoC-SwiGLU (top-k channel masking) Trainium2 Bass kernel.

out = (topk_mask(silu(x@Wg.T) * (x@Wu.T), k=1024 by |z|)) @ Wd.T

Strategy: data-parallel over tokens across 8 NeuronCores. Host pre-transposes
and casts operands to fp16 (full PE speed, ~2.3x less quantization noise than
bf16 - selection flips near the top-k threshold dominate the error budget).
Per 128-token tile the top-k threshold comes from a 2-step Newton iteration
on the count function, t <- t*(1 + (count(|z|>=t)-K)/G), seeded at
1.0559*mean|z|; counting uses fused compare+reduce ops with tokens on
partitions, alternating tiles between DVE and ACT (Sign-with-bias trick) with
their per-step scalar updates interleaved so the two chains stay parallel.
The masked z is transposed by the DMA xbar (one dma_start_transpose per
tile; no PE identity matmuls, no PSUM->SBUF copies) in a 3-stage pipeline:
search, transpose-kick one superblock later (so the kickoff never blocks the
ACT queue), down-projection one superblock after that. DMA kickoffs are
spread across queues (weights on sync, x/Wd on gpsimd SWDGE, out on scalar)
because each engine queue is strictly in-order and a single late wait
head-of-line-blocks everything behind it.
"""

import numpy as np
import ml_dtypes

import concourse.bass as bass
import concourse.bacc as bacc
import concourse.mybir as mybir
import concourse.tile as tile
from concourse import masks
from concourse.bass_utils import run_bass_kernel_spmd

FP32 = mybir.dt.float32
FP16 = mybir.dt.float16
BF16 = mybir.dt.bfloat16
FP8 = mybir.dt.float8e4

# Problem geometry (full problem, hardcoded per the harness contract)
B, S, D = 4, 4096, 1024
F = 4096
K_ACTIVE = 1024
N_CORES = 8
TOKENS = B * S                    # 16384
TOK_CORE = TOKENS // N_CORES      # 2048


def _build_nc(tok_core=TOK_CORE, d=D, f=F, k_active=K_ACTIVE, sb=256, fb=512,
              niter=2, g_slope=1100.0, zmask2=None, debug=False,
              act_mod=2, act_rem=(0,),
              z_bufs=4, absz_bufs=2, zm_bufs=2, zt_bufs=2, w_bufs=4, x_bufs=2,
              out_bufs=1, s_bufs=3, gu_bufs=6, dn_bufs=2,
              init_lo=0.82 * 1.0559, init_hi=1.18 * 1.0559,
              delay_tiles=2, ind_bufs=1, wd_chunks=4,
              repeat=1):
    n_dc = d // 128
    n_fc = f // 128
    n_fb = f // fb
    n_sb = tok_core // sb
    tps = sb // 128

    nc = bacc.Bacc("TRN2", target_bir_lowering=False, debug=False)
    xT = nc.declare_dram_parameter("xT", [d, tok_core], FP16, isOutput=False)
    WgT = nc.declare_dram_parameter("WgT", [d, f], FP16, isOutput=False)
    WuT = nc.declare_dram_parameter("WuT", [d, f], FP16, isOutput=False)
    WdT = nc.declare_dram_parameter("WdT", [f, d], FP16, isOutput=False)
    out = nc.declare_dram_parameter("out", [tok_core, d], FP32, isOutput=True)
    if debug:
        z_dbg = nc.declare_dram_parameter("z_dbg", [tok_core, f], FP32, isOutput=True)
        lo_dbg = nc.declare_dram_parameter("lo_dbg", [tok_core, 1], FP32, isOutput=True)
        zm_dbg = nc.declare_dram_parameter("zm_dbg", [tok_core, f], FP32, isOutput=True)

    xT_r = xT.rearrange("(c p) t -> p c t", p=128)     # [128, n_dc, tok_core]
    WgT_r = WgT.rearrange("(c p) f -> p c f", p=128)   # [128, n_dc, f]
    WuT_r = WuT.rearrange("(c p) f -> p c f", p=128)
    WdT_r = WdT.rearrange("(c p) d -> p c d", p=128)   # [128, n_fc, d]

    with tile.TileContext(nc) as tc:
        with (
            tc.tile_pool(name="const", bufs=1) as const_pool,
            tc.tile_pool(name="wd", bufs=1) as wd_pool,
            tc.tile_pool(name="xs", bufs=x_bufs) as x_pool,
            tc.tile_pool(name="wgu", bufs=w_bufs) as w_pool,
            tc.tile_pool(name="zb", bufs=z_bufs) as z_pool,
            tc.tile_pool(name="absz", bufs=absz_bufs) as absz_pool,
            tc.tile_pool(name="zm", bufs=zm_bufs) as zm_pool,
            tc.tile_pool(name="indp", bufs=ind_bufs) as ind_pool,
            tc.tile_pool(name="ztr", bufs=zt_bufs) as zt_pool,
            tc.tile_pool(name="silu", bufs=s_bufs) as s_pool,
            tc.tile_pool(name="outp", bufs=out_bufs) as out_pool,
            tc.tile_pool(name="small", bufs=4) as sm_pool,
            tc.tile_pool(name="gu_ps", bufs=gu_bufs, space="PSUM") as gu_psum,
            tc.tile_pool(name="dn_ps", bufs=dn_bufs, space="PSUM") as dn_psum,
        ):
            wd_sb = wd_pool.tile([128, n_fc, d], FP16, tag="wd")
            wd_issued = 0
            fc_per_chunk = n_fc // wd_chunks
            if repeat > 1:
                nc.sync.dma_start(wd_sb[:], WdT_r[:])
                wd_issued = wd_chunks
                rep_cm = tc.For_i(0, repeat, 1)
                rep_cm.__enter__()

            tile_idx = 0
            searched = []
            transposed = []
            x_next = None
            for isb in range(n_sb):
                if x_next is not None:
                    x_sb = x_next
                else:
                    x_sb = x_pool.tile([128, n_dc, sb], FP16, tag="x")
                    # first block: sync queue ahead of all weight kickoffs,
                    # split so the first LDWEIGHTS waits on half the data
                    h = n_dc // 2
                    nc.sync.dma_start(x_sb[:, :h, :],
                                      xT_r[:, :h, isb * sb:(isb + 1) * sb])
                    nc.sync.dma_start(x_sb[:, h:, :],
                                      xT_r[:, h:, isb * sb:(isb + 1) * sb])

                z_tiles = [z_pool.tile([128, f], FP16, tag="z", name=f"z_{isb}_{i}")
                           for i in range(tps)]

                for ifb in range(n_fb):
                    wg_t = w_pool.tile([128, n_dc, fb], FP16, tag="w")
                    nc.sync.dma_start(wg_t[:], WgT_r[:, :, ifb * fb:(ifb + 1) * fb])
                    wu_t = w_pool.tile([128, n_dc, fb], FP16, tag="w")
                    nc.sync.dma_start(wu_t[:], WuT_r[:, :, ifb * fb:(ifb + 1) * fb])
                    gfb = isb * n_fb + ifb
                    if wd_issued < wd_chunks and gfb >= 8 and gfb % 2 == 0:
                        # Wd in chunks, spread past the startup-critical phase
                        # (HBM is weight-stream-bound for the first ~2
                        # superblocks); first use is ~2 superblocks in
                        ck = wd_issued
                        nc.gpsimd.dma_start(
                            wd_sb[:, ck * fc_per_chunk:(ck + 1) * fc_per_chunk, :],
                            WdT_r[:, ck * fc_per_chunk:(ck + 1) * fc_per_chunk, :])
                        wd_issued += 1
                    if ifb == 4 and isb + 1 < n_sb:
                        # prefetch next superblock's x mid-block so its first
                        # LDWEIGHTS never waits
                        x_next = x_pool.tile([128, n_dc, sb], FP16,
                                             tag="x")
                        nc.gpsimd.dma_start(
                            x_next[:], xT_r[:, :, (isb + 1) * sb:(isb + 2) * sb])

                    for tt in range(tps):
                        xw = x_sb[:, :, tt * 128:(tt + 1) * 128]
                        g_ps = gu_psum.tile([128, fb], FP32, tag="gu")
                        u_ps = gu_psum.tile([128, fb], FP32, tag="gu")
                        for dc in range(n_dc):
                            nc.tensor.matmul(g_ps[:], xw[:, dc, :], wg_t[:, dc, :],
                                             start=(dc == 0), stop=(dc == n_dc - 1))
                        for dc in range(n_dc):
                            nc.tensor.matmul(u_ps[:], xw[:, dc, :], wu_t[:, dc, :],
                                             start=(dc == 0), stop=(dc == n_dc - 1))
                        s_t = s_pool.tile([128, fb], FP16, tag="s")
                        nc.scalar.activation(s_t[:], g_ps[:],
                                             mybir.ActivationFunctionType.Silu)
                        nc.vector.tensor_tensor(
                            z_tiles[tt][:, ifb * fb:(ifb + 1) * fb],
                            s_t[:], u_ps[:], mybir.AluOpType.mult)

                def emit_search_group(z_list, tile_idx0):
                    """Search all tiles of this superblock with their Newton
                    steps interleaved across engines: ACT tiles run Sign on
                    the scalar engine, DVE tiles run the fused compare+reduce,
                    and the per-step smalls interleave on DVE so neither
                    tile's chain serializes behind the other's big ops."""
                    n = len(z_list)
                    on_act = [((tile_idx0 + j) % act_mod) in act_rem
                              for j in range(n)]
                    # ACT tiles' abs first: their Sign chains start earliest
                    order = sorted(range(n), key=lambda j: 0 if on_act[j] else 1)
                    st = [None] * n
                    for j in order:
                        absz = absz_pool.tile([128, f], FP16, tag="absz")
                        s1 = sm_pool.tile([128, 1], FP32, tag="s1")
                        nc.scalar.activation(absz[:], z_list[j][:],
                                             mybir.ActivationFunctionType.Abs,
                                             accum_out=s1[:, 0:1])
                        st[j] = {"absz": absz, "s1": s1}
                    for j in order:
                        lo = sm_pool.tile([128, 1], FP32, tag="lo")
                        dd = sm_pool.tile([128, 1], FP32, tag="dd")
                        cnt = sm_pool.tile([128, 1], FP32, tag="cnt")
                        sgn = -1.0 if on_act[j] else 1.0
                        nc.vector.tensor_scalar_mul(
                            lo[:], st[j]["s1"][:],
                            sgn * (init_lo + init_hi) / 2 / f)
                        ind = ind_pool.tile([128, f],
                                            FP8 if on_act[j] else FP16,
                                            tag="ind_a" if on_act[j] else "ind_v")
                        st[j].update(lo=lo, dd=dd, cnt=cnt, ind=ind)
                    # Newton: t <- t * (1 + (count(|z|>=t) - K)/G).
                    # ACT path tracks -t (Sign bias) and counts 2c - F.
                    for it in range(niter):
                        for j in order:
                            s = st[j]
                            if on_act[j]:
                                nc.scalar.activation(
                                    s["ind"][:], s["absz"][:],
                                    mybir.ActivationFunctionType.Sign,
                                    bias=s["lo"][:, 0:1],
                                    accum_out=s["cnt"][:, 0:1])
                            else:
                                nc.vector.tensor_scalar(
                                    s["ind"][:], s["absz"][:], s["lo"][:, 0:1],
                                    None, mybir.AluOpType.is_ge,
                                    mybir.AluOpType.add,
                                    accum_out=s["cnt"][:, 0:1])
                        for j in order:
                            s = st[j]
                            if on_act[j]:
                                nc.vector.tensor_scalar(
                                    s["dd"][:], s["cnt"][:],
                                    float(f - 2 * k_active),
                                    1.0 / (2 * g_slope),
                                    mybir.AluOpType.add, mybir.AluOpType.mult)
                            else:
                                nc.vector.tensor_scalar(
                                    s["dd"][:], s["cnt"][:], float(-k_active),
                                    1.0 / g_slope,
                                    mybir.AluOpType.add, mybir.AluOpType.mult)
                            nc.vector.tensor_single_scalar(
                                s["dd"][:], s["dd"][:], 1.0,
                                mybir.AluOpType.add)
                            nc.vector.tensor_tensor(
                                s["lo"][:], s["lo"][:], s["dd"][:],
                                mybir.AluOpType.mult)
                    res = [None] * n
                    for j in order:
                        s = st[j]
                        if on_act[j]:
                            nc.vector.tensor_scalar_mul(s["lo"][:], s["lo"][:],
                                                        -1.0)
                        zmask = zm_pool.tile([128, f], FP16, tag="zm")
                        if on_act[j]:
                            nc.vector.scalar_tensor_tensor(
                                zmask[:], s["absz"][:], s["lo"][:, 0:1],
                                z_list[j][:], mybir.AluOpType.is_ge,
                                mybir.AluOpType.mult)
                        else:
                            # 2-op fast path: 4x packed compare + 2x packed mult
                            nc.vector.tensor_scalar(
                                s["ind"][:], s["absz"][:], s["lo"][:, 0:1],
                                None, mybir.AluOpType.is_ge)
                            nc.vector.tensor_tensor(
                                zmask[:], s["ind"][:], z_list[j][:],
                                mybir.AluOpType.mult)
                        res[j] = (zmask, s["lo"], z_list[j])
                    return res

                def emit_td(zt_t, lo, z_t, tok0):
                    # down-projection: out[t, :] = sum_f zmask[t, f] * WdT[f, :]
                    out_t = out_pool.tile([128, d], FP32, tag="out")
                    dbw = min(512, d)
                    for db in range(d // dbw):
                        dn_ps = dn_psum.tile([128, dbw], FP32, tag="dn")
                        for c in range(n_fc):
                            nc.tensor.matmul(dn_ps[:], zt_t[:, c, :],
                                             wd_sb[:, c, db * dbw:(db + 1) * dbw],
                                             start=(c == 0), stop=(c == n_fc - 1))
                        nc.scalar.activation(out_t[:, db * dbw:(db + 1) * dbw],
                                             dn_ps[:],
                                             mybir.ActivationFunctionType.Copy)

                    # scalar-queue kickoff lands right after the producing
                    # copy with no cross-engine wait, and stays off the
                    # weight-stream queue
                    nc.scalar.dma_start(out[tok0:tok0 + 128, :], out_t[:])
                    if debug:
                        nc.sync.dma_start(lo_dbg[tok0:tok0 + 128, :], lo[:])
                        nc.gpsimd.dma_start(z_dbg[tok0:tok0 + 128, :], z_t[:])

                # three-stage pipeline, one superblock apart each: kick
                # the xbar transposes for last superblock's zmasks (complete
                # by now, so the kickoff never blocks the ACT queue; placed
                # here, after the silus, their xbar traffic also stays out of
                # the weight stream's bandwidth-critical phase), down-project
                # the tiles transposed one superblock before that, then
                # search this superblock.
                for (zmask_, lo_, zt_ref_, tok0_) in searched:
                    ztt = zt_pool.tile([128, n_fc, 128], FP16, tag="zt")
                    nc.scalar.dma_start_transpose(ztt[:], zmask_[:])
                    transposed.append((ztt, lo_, zt_ref_, tok0_))
                searched = []
                while transposed and transposed[0][3] < (isb - 1) * sb:
                    (ztt, lo_, zt_ref_, tok0_) = transposed.pop(0)
                    emit_td(ztt, lo_, zt_ref_, tok0_)
                for tt, ctx_ in enumerate(emit_search_group(z_tiles, tile_idx)):
                    searched.append((*ctx_, isb * sb + tt * 128))
                tile_idx += tps
            while transposed:
                (ztt, lo_, zt_ref_, tok0_) = transposed.pop(0)
                emit_td(ztt, lo_, zt_ref_, tok0_)
            for (zmask_, lo_, zt_ref_, tok0_) in searched:
                ztt = zt_pool.tile([128, n_fc, 128], FP16, tag="zt")
                nc.scalar.dma_start_transpose(ztt[:], zmask_[:])
                emit_td(ztt, lo_, zt_ref_, tok0_)
            if repeat > 1:
                rep_cm.__exit__(None, None, None)
    nc.compile()
    return nc


_NC_CACHE = {}

# test-harness hooks (not used by the grading path)
TRACE = False
TRACE_KWARGS = {}
LAST_RESULT = None
BUILD_KWARGS = {}


def _get_nc(**kw):
    key = tuple(sorted(kw.items()))
    if key not in _NC_CACHE:
        _NC_CACHE[key] = _build_nc(**kw)
    return _NC_CACHE[key]


def kernel(x, Wg, Wu, Wd):
    xf = np.ascontiguousarray(x, dtype=np.float32).reshape(TOKENS, D)
    bf = np.float16
    WgT = np.ascontiguousarray(Wg.T).astype(bf)
    WuT = np.ascontiguousarray(Wu.T).astype(bf)
    WdT = np.ascontiguousarray(Wd.T).astype(bf)

    in_maps = []
    for c in range(N_CORES):
        xs = xf[c * TOK_CORE:(c + 1) * TOK_CORE]
        in_maps.append({
            "xT": np.ascontiguousarray(xs.T).astype(bf),
            "WgT": WgT, "WuT": WuT, "WdT": WdT,
        })

    nc = _get_nc(**BUILD_KWARGS)
    res = run_bass_kernel_spmd(nc, in_maps, core_ids=list(range(N_CORES)),
                               trace=TRACE, **TRACE_KWARGS)
    global LAST_RESULT
    LAST_RESULT = res
    out = np.concatenate([res.results[c]["out"] for c in range(N_CORES)], axis=0)
    return out.reshape(B, S, D)
